# revision 1
# baseline (speedup 1.0000x reference)
"""LSTM discriminator kernel v2 — Trainium2, 8 cores SPMD, data-parallel batch.

Differences vs v1:
  - NG independent batch groups per core (default 2), software-pipelined so
    engine handoff latency of one group hides under the other's work.
  - Recurrence matmul: K = NCH_G*H + 1 with a ones row (bias) and
    zero-padded per-chunk weight variants, so one transposed-h tile serves
    all chunks of a group with a single PSUM->SBUF copy.
  - c-state stored as c2 = 2c so both activations are plain sigmoid scale=1.
  - t = f*c2 runs on the Pool (gpsimd) engine to offload DVE.

Per-core, per-step, per-group (group = NCH_G chunks of 128 batch):
  PE : NCH_G feed matmuls (stationary feed chunk, FWL), NCH_G rec matmuls
       (stationary HT_g [K,128], moving zero-padded whh variant), 1 transpose
       (hbm [128, NCH_G*H] -> [NCH_G*H, 128] PSUM).
  ACT: acts = sigmoid(gates [128, NCH_G*G]) PSUM->SBUF bf16;
       csig = sigmoid(c2 [128, NCH_G*H]) -> bf16.
  DVE: w = (g-0.5)*i; c2 = 4w + t; hbm = (csig-0.5)*o; copy hT_ps -> HT_g.
  Pool: t = f * c2_old.
"""

import numpy as np
import ml_dtypes

import concourse.bass as bass
import concourse.mybir as mybir
from concourse.tile import TileContext
from concourse.bass_utils import run_bass_kernel_spmd

F32 = mybir.dt.float32
BF16 = mybir.dt.bfloat16
BF = ml_dtypes.bfloat16

B, T, D, H = 4096, 256, 128, 32
G = 4 * H
NCORES = 8
BC = B // NCORES          # 512 batch rows per core
NCH = BC // 128           # 4 chunks of 128
TCHUNK = 8                # timesteps per feed DMA

NG = 2                    # independent pipelined groups per core
DBG = False               # dump HT + c2 state at end
REP = 1                   # timing probe: repeat the T-loop (breaks numerics)
GBUFS = 2                 # gates PSUM pool double-buffering
T_POOL = False            # t = f*c2 on Pool engine
HBM_POOL = False          # hbm op on Pool engine
SPLIT_ACTS = True         # per-group acts call (False: one wide call)

SIG = mybir.ActivationFunctionType.Sigmoid
MULT = mybir.AluOpType.mult
SUB = mybir.AluOpType.subtract
ADD = mybir.AluOpType.add

LAST_RESULTS = None

_lgl_ctr = [0]


def _legalize_sync_waits(nc):
    for fn in nc.m.functions:
        for blk in fn.blocks:
            new = []
            changed = False
            for inst in blk.instructions:
                si = getattr(inst, "sync_info", None)
                waits = list(si.on_wait) if (si is not None and si.on_wait) else []
                if len(waits) > 1:
                    for w in waits[:-1]:
                        _lgl_ctr[0] += 1
                        new.append(mybir.InstNoOp(
                            name=f"I-lgl-{_lgl_ctr[0]}",
                            engine=inst.engine,
                            sync_info=mybir.SyncInfo(on_wait=[w], on_update=[]),
                            bass_nofuse=True,
                        ))
                    si.on_wait = waits[-1:]
                    changed = True
                new.append(inst)
            if changed:
                blk.instructions[:] = new


_nc_cache = None


def _build_nc():
    nch_g = NCH // NG          # chunks per group
    gb = 128 * nch_g           # batch per group
    kk = nch_g * H + 1         # contraction depth of rec matmul (ones row last)
    hw = nch_g * H             # h width per group

    nc = bass.Bass()

    feedT8 = nc.dram_tensor("feedT8", [T // TCHUNK, D, TCHUNK * BC], BF16,
                            kind="ExternalInput")
    wihT = nc.dram_tensor("wihT", [D, G], BF16, kind="ExternalInput")
    # zero-padded whh variants: for chunk c, rows [c*H, (c+1)*H) = whh rows,
    # all other unit rows zero, last row = bias.
    whh_e = nc.dram_tensor("whh_e", [nch_g, kk, G], BF16, kind="ExternalInput")
    wout_e = nc.dram_tensor("wout_e", [nch_g, kk, 2], BF16, kind="ExternalInput")
    h0T = nc.dram_tensor("h0T", [kk, NG * 128], BF16, kind="ExternalInput")
    c0bm = nc.dram_tensor("c0bm", [128, NG * hw], F32, kind="ExternalInput")
    ident_d = nc.dram_tensor("ident_d", [128, 128], BF16, kind="ExternalInput")
    y_out = nc.dram_tensor("y_out", [2, BC], F32, kind="ExternalOutput")
    if DBG:
        ht_dump = nc.dram_tensor("ht_dump", [NG, kk, 128], BF16,
                                 kind="ExternalOutput")
        c2_dump = nc.dram_tensor("c2_dump", [128, NG * hw], F32,
                                 kind="ExternalOutput")
        acts_dump = nc.dram_tensor("acts_dump", [NG, 128, nch_g * G], BF16,
                                   kind="ExternalOutput")

    with TileContext(nc) as tc:
        with (
            tc.tile_pool(name="const", bufs=1) as cpool,
            tc.tile_pool(name="state", bufs=1) as spool,
            tc.tile_pool(name="feed", bufs=2) as fpool,
        ):
            wihT_sb = cpool.tile([D, G], BF16, tag="wihT")
            whh_sb = [cpool.tile([kk, G], BF16, tag=f"whh{c}", name=f"whh_sb{c}") for c in range(nch_g)]
            wout_sb = [cpool.tile([kk, 2], BF16, tag=f"wout{c}", name=f"wout_sb{c}") for c in range(nch_g)]
            ident = cpool.tile([128, 128], BF16, tag="ident")
            HT = [spool.tile([kk, 128], BF16, tag=f"HT{g}", name=f"HT_{g}") for g in range(NG)]
            c2 = [spool.tile([128, hw], F32, tag=f"c2_{g}", name=f"c2t_{g}") for g in range(NG)]

            nc.sync.dma_start(wihT_sb[:], wihT[:])
            for c in range(nch_g):
                nc.sync.dma_start(whh_sb[c][:], whh_e[c])
                nc.sync.dma_start(wout_sb[c][:], wout_e[c])
            nc.sync.dma_start(ident[:], ident_d[:])
            for g in range(NG):
                nc.sync.dma_start(HT[g][:], h0T[:, g * 128:(g + 1) * 128])
                nc.sync.dma_start(c2[g][:], c0bm[:, g * hw:(g + 1) * hw])

            wpool = [tc.alloc_tile_pool(name=f"wk{g}", bufs=2)
                     for g in range(NG)]
            gpool = [tc.alloc_tile_pool(name=f"gp{g}", bufs=GBUFS, space="PSUM")
                     for g in range(NG)]
            hpool_s = tc.alloc_tile_pool(name="hp", bufs=1, space="PSUM")
            hpool = [hpool_s for _ in range(NG)]

            eng_t = nc.gpsimd if T_POOL else nc.vector
            eng_h = nc.gpsimd if HBM_POOL else nc.vector

            fbuf = None
            for t in range(REP * T):
                tb, ts = divmod(t % T, TCHUNK)
                if ts == 0:
                    fbuf = fpool.tile([D, TCHUNK * BC], BF16, tag="fbuf")
                    nc.sync.dma_start(fbuf[:], feedT8[tb])
                feed_t = fbuf[:, ts * BC:(ts + 1) * BC]

                gates = []
                for g in range(NG):
                    gt = gpool[g].tile([128, nch_g * G], F32, tag=f"gates{g}", name=f"gates_{g}")
                    gates.append(gt)
                    for c in range(nch_g):
                        bsl = slice((g * nch_g + c) * 128, (g * nch_g + c + 1) * 128)
                        nc.tensor.matmul(gt[:, c * G:(c + 1) * G],
                                         feed_t[:, bsl], wihT_sb[:],
                                         start=True, stop=False)
                        nc.tensor.matmul(gt[:, c * G:(c + 1) * G],
                                         HT[g][:], whh_sb[c][:],
                                         start=False, stop=True)

                acts = []
                if SPLIT_ACTS:
                    for g in range(NG):
                        at = wpool[g].tile([128, nch_g * G], BF16, tag=f"acts{g}", name=f"acts_{g}")
                        nc.scalar.activation(at[:], gates[g][:], SIG)
                        acts.append(at)
                else:
                    # one wide ACT call: requires groups' gates adjacent; use
                    # per-group calls anyway when tiles aren't contiguous.
                    for g in range(NG):
                        at = wpool[g].tile([128, nch_g * G], BF16, tag=f"acts{g}", name=f"acts_{g}")
                        nc.scalar.activation(at[:], gates[g][:], SIG)
                        acts.append(at)

                wts, tts = [], []
                for g in range(NG):
                    av = acts[g][:].rearrange("p (c g) -> p c g", c=nch_g)
                    i_sl = av[:, :, 0:32]
                    f_sl = av[:, :, 32:64]
                    g_sl = av[:, :, 64:96]
                    cv = c2[g][:].rearrange("p (c h) -> p c h", c=nch_g)

                    w_t = wpool[g].tile([128, hw], BF16, tag=f"w{g}", name=f"wt_{g}")
                    wv = w_t[:].rearrange("p (c h) -> p c h", c=nch_g)
                    nc.vector.scalar_tensor_tensor(wv, g_sl, 0.5, i_sl, SUB, MULT)
                    wts.append(w_t)

                    t_t = wpool[g].tile([128, hw], F32, tag=f"t{g}", name=f"tt_{g}")
                    tv = t_t[:].rearrange("p (c h) -> p c h", c=nch_g)
                    eng_t.tensor_tensor(tv, f_sl, cv, MULT)
                    tts.append(t_t)

                for g in range(NG):
                    # c2 = 4*w + t   (c2 == 2c)
                    nc.vector.scalar_tensor_tensor(
                        c2[g][:], wts[g][:], 4.0, tts[g][:], MULT, ADD)

                sigs = []
                for g in range(NG):
                    sg = wpool[g].tile([128, hw], BF16, tag=f"sig{g}", name=f"sig_{g}")
                    nc.scalar.activation(sg[:], c2[g][:], SIG)
                    sigs.append(sg)

                hbms = []
                for g in range(NG):
                    av = acts[g][:].rearrange("p (c g) -> p c g", c=nch_g)
                    o_sl = av[:, :, 96:128]
                    hb = wpool[g].tile([128, hw], BF16, tag=f"hbm{g}", name=f"hbm_{g}")
                    hv = hb[:].rearrange("p (c h) -> p c h", c=nch_g)
                    sv = sigs[g][:].rearrange("p (c h) -> p c h", c=nch_g)
                    eng_h.scalar_tensor_tensor(hv, sv, 0.5, o_sl, SUB, MULT)
                    hbms.append(hb)

                npair = (NG + 1) // 2
                ht_t = [hpool_s.tile([min(2, NG) * hw, 128], BF16,
                                     tag=f"hT{p}", name=f"hTp_{p}")
                        for p in range(npair)]
                for g in range(NG):
                    sl = ht_t[g // 2][(g % 2) * hw:(g % 2 + 1) * hw, :]
                    nc.tensor.transpose(sl, hbms[g][:], ident[:])

                for g in range(NG):
                    sl = ht_t[g // 2][(g % 2) * hw:(g % 2 + 1) * hw, :]
                    nc.vector.tensor_copy(HT[g][0:hw, :], sl)

            if DBG:
                for g in range(NG):
                    nc.sync.dma_start(ht_dump[g], HT[g][:])
                    nc.sync.dma_start(c2_dump[:, g * hw:(g + 1) * hw], c2[g][:])
                    nc.sync.dma_start(acts_dump[g], acts[g][:])

            y_ps = hpool[0].tile([2, BC], F32, tag="y")
            for g in range(NG):
                for c in range(nch_g):
                    bsl = slice((g * nch_g + c) * 128, (g * nch_g + c + 1) * 128)
                    nc.tensor.matmul(y_ps[:, bsl], wout_sb[c][:], HT[g][:],
                                     start=True, stop=True)
            y_sb = wpool[0].tile([2, BC], F32, tag="ysb")
            nc.scalar.copy(y_sb[:], y_ps[:])
            nc.sync.dma_start(y_out[:], y_sb[:])

            hpool_s.release()
            for g in reversed(range(NG)):
                gpool[g].release()
            for g in reversed(range(NG)):
                wpool[g].release()

    _legalize_sync_waits(nc)
    return nc


# -------------------------------------------------------------------- host ---
def _prep_core_inputs(feed_c, W_ih, W_hh, b_ih, b_hh, W_out, b_out, h0_c, c0_c):
    nch_g = NCH // NG
    kk = nch_g * H + 1
    hw = nch_g * H
    g_rows = slice(64, 96)  # PyTorch gate order i,f,g,o

    wih_p = W_ih.astype(np.float32).copy()
    wih_p[g_rows] *= 2.0
    wihT = np.ascontiguousarray(wih_p.T).astype(BF)

    whh_p = (2.0 * W_hh.astype(np.float32)).copy()
    whh_p[g_rows] *= 2.0
    bias = (b_ih + b_hh).astype(np.float32).copy()
    bias[g_rows] *= 2.0
    whhT = whh_p.T  # [H, G]

    whh_e = np.zeros((nch_g, kk, G), np.float32)
    wout_e = np.zeros((nch_g, kk, 2), np.float32)
    woutT = 2.0 * W_out.astype(np.float32).T  # [H, 2]
    for c in range(nch_g):
        whh_e[c, c * H:(c + 1) * H] = whhT
        whh_e[c, hw] = bias
        wout_e[c, c * H:(c + 1) * H] = woutT
        wout_e[c, hw] = b_out.astype(np.float32)

    # feed_c [BC, T, D] -> [T/8, D, 8*BC]
    ft = feed_c.transpose(1, 2, 0).reshape(T // TCHUNK, TCHUNK, D, BC)
    feedT8 = np.ascontiguousarray(ft.transpose(0, 2, 1, 3)).reshape(
        T // TCHUNK, D, TCHUNK * BC).astype(BF)

    # h0T [kk, NG*128]: rows c*H+h for group g col p = h0(g*gb + c*128 + p)/2
    h0T = np.ones((kk, NG * 128), np.float32)
    hh = h0_c.astype(np.float32) / 2.0  # [BC, H]
    hh4 = hh.reshape(NG, nch_g, 128, H)  # [g, c, p, h]
    for g in range(NG):
        h0T[0:hw, g * 128:(g + 1) * 128] = (
            hh4[g].transpose(0, 2, 1).reshape(hw, 128))
    h0T = h0T.astype(BF)

    # c0bm [128, NG*hw]: col g*hw + c*H + h, row p -> 2*c0(g*gb+c*128+p, h)
    c4 = 2.0 * c0_c.astype(np.float32).reshape(NG, nch_g, 128, H)  # [g,c,p,h]
    c0bm = np.ascontiguousarray(
        c4.transpose(2, 0, 1, 3).reshape(128, NG * hw)).astype(np.float32)

    ident = np.eye(128, dtype=np.float32).astype(BF)

    return dict(feedT8=feedT8, wihT=wihT, whh_e=whh_e.astype(BF),
                wout_e=wout_e.astype(BF), h0T=h0T, c0bm=c0bm, ident_d=ident)


def kernel(feed, W_ih, W_hh, b_ih, b_hh, W_out, b_out, h0, c0):
    global _nc_cache, LAST_RESULTS
    feed = np.asarray(feed, dtype=np.float32)
    W_ih = np.asarray(W_ih, dtype=np.float32)
    W_hh = np.asarray(W_hh, dtype=np.float32)
    b_ih = np.asarray(b_ih, dtype=np.float32)
    b_hh = np.asarray(b_hh, dtype=np.float32)
    W_out = np.asarray(W_out, dtype=np.float32)
    b_out = np.asarray(b_out, dtype=np.float32)
    h0 = np.asarray(h0, dtype=np.float32)
    c0 = np.asarray(c0, dtype=np.float32)

    if _nc_cache is None:
        _nc_cache = _build_nc()
    nc = _nc_cache

    in_maps = []
    for c in range(NCORES):
        rows = slice(c * BC, (c + 1) * BC)
        in_maps.append(_prep_core_inputs(
            feed[rows], W_ih, W_hh, b_ih, b_hh, W_out, b_out,
            h0[rows], c0[rows]))

    res = run_bass_kernel_spmd(nc, in_maps, core_ids=list(range(NCORES)))
    LAST_RESULTS = res

    out = np.empty((B, 2), dtype=np.float32)
    for c in range(NCORES):
        out[c * BC:(c + 1) * BC] = res.results[c]["y_out"].T
    return out



# revision 2
# speedup vs baseline: 1.5778x; 1.5778x over previous
"""LSTM discriminator kernel v3 — Trainium2, 8 cores SPMD, data-parallel.

Packed gate-transposed layout. Per core (BC=512 batch): Q=2 phase-offset
streams x (P=4 sub-streams packed on partitions) x NS=64 columns. State
tiles are [128, NS] with partition 32j+u = (sub-stream j, hidden unit u).

Per stream per step:
  PE : 4 rec MMs (lhsT = blockdiag(2*Whh_g.T) [128,128], rhs = hsT [128,NS])
       close the gates PSUM bank [128, 4*NS] (gate g at cols g*NS); prefill
       of step t+1's bank: 1 bias MM (K=4 indicator) + 16 feed MMs
       (lhsT = blockdiag(Wih_g[:,dchunk q].T), rhs = repacked feed [128,NS]).
  ACT: acts = sigmoid(gates) [128, 4*NS] PSUM->SBUF bf16; csig = sigmoid(cs).
  DVE: w = (sg-0.5)*i ; t = f*cs ; cs = 4w + t (fp32) ;
       hsT = (csig-0.5)*o  -> directly the next rec MM rhs (no transpose).

Scaling: hs = h/2, cs = 2c, g-gate rows of W/bias x2 => all sigmoid.
"""

import numpy as np
import ml_dtypes

import concourse.bass as bass
import concourse.mybir as mybir
from concourse.tile import TileContext
from concourse.bass_utils import run_bass_kernel_spmd

F32 = mybir.dt.float32
BF16 = mybir.dt.bfloat16
BF = ml_dtypes.bfloat16

B, T, D, H = 4096, 256, 128, 32
NCORES = 8
BC = B // NCORES          # 512
Q = 2                     # phase-offset streams
P = 4                     # sub-streams packed on partitions
NS = BC // (Q * P)        # 64 columns per stream
GW = 512                  # gates tile width: full 2KB PSUM bank (fp32 cols)
TCHUNK = 8

REP = 1                   # timing probe: repeat T-loop (breaks numerics)
LEGALIZE = True           # split multi-waits (needed for HW)

SIG = mybir.ActivationFunctionType.Sigmoid
IDENT = mybir.ActivationFunctionType.Identity
MULT = mybir.AluOpType.mult
SUB = mybir.AluOpType.subtract
ADD = mybir.AluOpType.add

ROLE = {}
_lgl_ctr = [0]
LAST_RESULTS = None

# role-prefix -> engine whose semaphore is the true data dependency; that
# wait stays on the instruction (pre-decoded, parks in the engine wait
# queue); stale waits move to NoOps that resolve instantly.
_KEEP_ENGINE = {
    "sig": mybir.EngineType.PE,
    "csig": mybir.EngineType.DVE,
    "hsT": mybir.EngineType.Activation,
    "rec": mybir.EngineType.DVE,
    "w": mybir.EngineType.Activation,
    "t": mybir.EngineType.Activation,
    "cs": mybir.EngineType.DVE,
}


def _tag(inst, role):
    ROLE[inst.ins.name] = role
    return inst


def _role_keep_engine(name):
    role = ROLE.get(name)
    if not role:
        return None
    for pre, eng in _KEEP_ENGINE.items():
        if role.startswith(pre):
            return eng
    return None


def _legalize_sync_waits(nc):
    sem_eng = {}
    for fn in nc.m.functions:
        for blk in fn.blocks:
            for inst in blk.instructions:
                si = getattr(inst, "sync_info", None)
                if si is not None and si.on_update:
                    for u in si.on_update:
                        sem_eng.setdefault(u.id, set()).add(inst.engine)

    def wait_engine(w):
        engs = sem_eng.get(w.id, set())
        return next(iter(engs)) if len(engs) == 1 else None

    for fn in nc.m.functions:
        for blk in fn.blocks:
            new = []
            changed = False
            for inst in blk.instructions:
                si = getattr(inst, "sync_info", None)
                waits = list(si.on_wait) if (si is not None and si.on_wait) else []
                if len(waits) > 1:
                    keep_idx = len(waits) - 1
                    ke = _role_keep_engine(inst.name)
                    if ke is not None:
                        for idx, w in enumerate(waits):
                            if wait_engine(w) == ke:
                                keep_idx = idx
                                break
                    keep = [waits[keep_idx]]
                    move = [w for idx, w in enumerate(waits) if idx != keep_idx]
                    for w in move:
                        _lgl_ctr[0] += 1
                        new.append(mybir.InstNoOp(
                            name=f"I-lgl-{_lgl_ctr[0]}",
                            engine=inst.engine,
                            sync_info=mybir.SyncInfo(on_wait=[w], on_update=[]),
                            bass_nofuse=True,
                        ))
                    si.on_wait = keep
                    changed = True
                new.append(inst)
            if changed:
                blk.instructions[:] = new


WP_WIH = 0
WP_WHH = WP_WIH + 16 * 128
WP_WOUT = WP_WHH + 4 * 128
WP_H0 = WP_WOUT + 4 * 2
WP_END = WP_H0 + Q * NS
SP_BIAS = 0
SP_IND = 128
SP_END = SP_IND + 4 * NS


def _build_nc():
    nc = bass.Bass()

    feedT = nc.dram_tensor("feedT", [T // TCHUNK, 128, TCHUNK * BC], BF16,
                           kind="ExternalInput")
    wpack = nc.dram_tensor("wpack", [128, WP_END], BF16, kind="ExternalInput")
    spack = nc.dram_tensor("spack", [4, SP_END], BF16, kind="ExternalInput")
    boutd = nc.dram_tensor("boutd", [2, 1], F32, kind="ExternalInput")
    c0T = nc.dram_tensor("c0T", [128, Q * NS], F32, kind="ExternalInput")
    y_out = nc.dram_tensor("y_out", [2, Q * P * NS], F32,
                           kind="ExternalOutput")

    with TileContext(nc) as tc:
        with (
            tc.tile_pool(name="const", bufs=1) as cpool,
            tc.tile_pool(name="state", bufs=1) as spool,
            tc.tile_pool(name="feed", bufs=4) as fpool,
        ):
            wp_sb = cpool.tile([128, WP_END], BF16, tag="wpack")
            sp_sb = cpool.tile([4, SP_END], BF16, tag="spack")
            bout_sb = cpool.tile([2, 1], F32, tag="bout")
            hs0_sb = spool.tile([128, Q * NS], BF16, tag="hsT")
            cs0_sb = spool.tile([128, Q * NS], F32, tag="cs")

            wih_sb = [[wp_sb[:, WP_WIH + (4 * g + q) * 128:
                             WP_WIH + (4 * g + q + 1) * 128]
                       for q in range(4)] for g in range(4)]
            whh_sb = [wp_sb[:, WP_WHH + g * 128:WP_WHH + (g + 1) * 128]
                      for g in range(4)]
            wout_sb = [wp_sb[:, WP_WOUT + j * 2:WP_WOUT + (j + 1) * 2]
                       for j in range(P)]
            bias4 = sp_sb[:, SP_BIAS:SP_BIAS + 128]
            ind4 = sp_sb[:, SP_IND:SP_IND + 4 * NS]
            hsT = [hs0_sb[:, s * NS:(s + 1) * NS] for s in range(Q)]
            cs = [cs0_sb[:, s * NS:(s + 1) * NS] for s in range(Q)]

            nc.sync.dma_start(wp_sb[:], wpack[:])
            nc.sync.dma_start(sp_sb[:], spack[:])
            nc.sync.dma_start(bout_sb[:], boutd[:])
            nc.sync.dma_start(cs0_sb[:], c0T[:])
            nc.sync.dma_start(hs0_sb[:], wpack[:, WP_H0:WP_END])

            wpool = [tc.alloc_tile_pool(name=f"wk{s}", bufs=2)
                     for s in range(Q)]
            gpool = [tc.alloc_tile_pool(name=f"gp{s}", bufs=2, space="PSUM")
                     for s in range(Q)]

            fbufs = {}

            def feed_dma(blk):
                fb = fpool.tile([128, TCHUNK * BC], BF16, tag="fbuf")
                nc.sync.dma_start(fb[:], feedT[blk])
                fbufs[blk] = fb

            def prefill(t, gts):
                tb, ts = divmod(t % T, TCHUNK)
                fb = fbufs[tb]
                for s in range(Q):
                    gt = gts[s]
                    mm = nc.tensor.matmul(gt[:, 0:4 * NS], bias4, ind4,
                                          start=True, stop=False)
                    mm.ins.bass_skip_group_check = True
                    for g in range(4):
                        for q in range(4):
                            col = ts * BC + s * (P * NS) + q * NS
                            mm = nc.tensor.matmul(
                                gt[:, g * NS:(g + 1) * NS],
                                wih_sb[g][q], fb[:, col:col + NS],
                                start=False, stop=False)
                            mm.ins.bass_skip_group_check = True

            feed_dma(0)
            feed_dma(1)
            g_cur = [gpool[s].tile([128, GW], F32, tag=f"g{s}",
                                   name=f"g{s}_init") for s in range(Q)]
            prefill(0, g_cur)

            for t in range(REP * T):
                tm = t % T
                if tm % TCHUNK == 0 and tm // TCHUNK + 2 < T // TCHUNK:
                    feed_dma(tm // TCHUNK + 2)

                for s in range(Q):
                    for g in range(4):
                        mm = nc.tensor.matmul(
                            g_cur[s][:, g * NS:(g + 1) * NS],
                            whh_sb[g], hsT[s], start=False, stop=True)
                        mm.ins.bass_skip_group_check = True
                        _tag(mm, f"rec{s}g{g}")

                acts = []
                for s in range(Q):
                    at = wpool[s].tile([128, 4 * NS], BF16, tag=f"acts{s}",
                                       name=f"acts{s}_{t}")
                    _tag(nc.scalar.activation(at[:], g_cur[s][:, 0:4 * NS],
                                              SIG), f"sig{s}")
                    acts.append(at)

                g_nxt = [gpool[s].tile([128, GW], F32, tag=f"g{s}",
                                       name=f"g{s}_{t + 1}") for s in range(Q)]
                prefill(t + 1, g_nxt)

                wts, tts = [], []
                for s in range(Q):
                    a = acts[s]
                    i_sl = a[:, 0:NS]
                    f_sl = a[:, NS:2 * NS]
                    sg_sl = a[:, 2 * NS:3 * NS]
                    w_t = wpool[s].tile([128, NS], BF16, tag=f"w{s}",
                                        name=f"w{s}_{t}")
                    _tag(nc.vector.scalar_tensor_tensor(w_t[:], sg_sl, 0.5,
                                                        i_sl, SUB, MULT),
                         f"w{s}")
                    t_t = wpool[s].tile([128, NS], F32, tag=f"t{s}",
                                        name=f"t{s}_{t}")
                    _tag(nc.vector.tensor_tensor(t_t[:], f_sl, cs[s], MULT),
                         f"t{s}")
                    wts.append(w_t)
                    tts.append(t_t)
                for s in range(Q):
                    _tag(nc.vector.scalar_tensor_tensor(cs[s], wts[s][:], 4.0,
                                                        tts[s][:], MULT, ADD),
                         f"cs{s}")

                csigs = []
                for s in range(Q):
                    cg = wpool[s].tile([128, NS], BF16, tag=f"csig{s}",
                                       name=f"csig{s}_{t}")
                    _tag(nc.scalar.activation(cg[:], cs[s], SIG), f"csig{s}")
                    csigs.append(cg)

                for s in range(Q):
                    o_sl = acts[s][:, 3 * NS:4 * NS]
                    _tag(nc.vector.scalar_tensor_tensor(hsT[s], csigs[s][:],
                                                        0.5, o_sl, SUB, MULT),
                         f"hsT{s}")

                g_cur = g_nxt

            # final linear: y = 2*hs @ Wout.T + b_out
            y_sb = wpool[0].tile([2, Q * P * NS], F32, tag="ysb", name="ysb")
            for s in range(Q):
                y_ps = gpool[s].tile([2, GW], F32, tag=f"g{s}",
                                     name=f"y_ps{s}")
                for j in range(P):
                    mm = nc.tensor.matmul(y_ps[:, j * NS:(j + 1) * NS],
                                          wout_sb[j], hsT[s],
                                          start=True, stop=True)
                    mm.ins.bass_skip_group_check = True
                nc.scalar.activation(y_sb[:, s * P * NS:(s + 1) * P * NS],
                                     y_ps[:, 0:P * NS], IDENT,
                                     bias=bout_sb[:])
            nc.sync.dma_start(y_out[:], y_sb[:])

            for s in reversed(range(Q)):
                gpool[s].release()
            for s in reversed(range(Q)):
                wpool[s].release()

    if LEGALIZE:
        _legalize_sync_waits(nc)
    return nc


_nc_cache = None


# ------------------------------------------------------------------- host ---
def _prep_core_inputs(feed_c, W_ih, W_hh, b_ih, b_hh, W_out, b_out,
                      h0_c, c0_c):
    """feed_c [BC, T, D]; h0_c/c0_c [BC, H]. Input map for one core."""
    g_rows = slice(64, 96)  # PyTorch gate order i,f,g,o

    wih_s = W_ih.astype(np.float32).copy()
    wih_s[g_rows] *= 2.0
    bias_s = (b_ih + b_hh).astype(np.float32).copy()
    bias_s[g_rows] *= 2.0
    whh_d = (2.0 * W_hh.astype(np.float32)).copy()
    whh_d[g_rows] *= 2.0

    # blockdiag over j of W_ih_s[gate g rows, d-chunk q].T [32,32]
    wihT = np.zeros((4, 4, 128, 128), np.float32)
    for g in range(4):
        for q in range(4):
            blk = wih_s[32 * g:32 * (g + 1), 32 * q:32 * (q + 1)].T
            for j in range(P):
                wihT[g, q, 32 * j:32 * (j + 1), 32 * j:32 * (j + 1)] = blk

    whhT = np.zeros((4, 128, 128), np.float32)
    for g in range(4):
        blk = whh_d[32 * g:32 * (g + 1), :].T  # [h, u]
        for j in range(P):
            whhT[g, 32 * j:32 * (j + 1), 32 * j:32 * (j + 1)] = blk

    woutT = np.zeros((P, 128, 2), np.float32)
    wo = 2.0 * W_out.astype(np.float32).T  # [h, 2]
    for j in range(P):
        woutT[j, 32 * j:32 * (j + 1), :] = wo

    # feed [BC, T, D] -> [T/8, 128, 8*BC]: row 32j+dd,
    # col ts*BC + s*256 + q*64 + n  ->  feed[s*256 + j*64 + n, t, 32q + dd]
    fx = feed_c.reshape(Q, P, NS, T, 4, 32)               # [s,j,n,t,q,dd]
    ft = fx.transpose(3, 1, 5, 0, 4, 2)                   # [t,j,dd,s,q,n]
    ft = ft.reshape(T // TCHUNK, TCHUNK, 128, BC)
    feedT = np.ascontiguousarray(ft.transpose(0, 2, 1, 3)).reshape(
        T // TCHUNK, 128, TCHUNK * BC).astype(BF)

    # h0T (hs=h/2, bf16) / c0T (cs=2c, f32): row 32j+u, col s*NS+n
    h4 = (h0_c.astype(np.float32) / 2.0).reshape(Q, P, NS, H)  # [s,j,n,u]
    h0T = np.ascontiguousarray(h4.transpose(1, 3, 0, 2).reshape(P * H, Q * NS))
    c4 = (2.0 * c0_c.astype(np.float32)).reshape(Q, P, NS, H)
    c0T = np.ascontiguousarray(c4.transpose(1, 3, 0, 2).reshape(P * H, Q * NS))

    wpack = np.concatenate(
        [wihT.reshape(16, 128, 128).transpose(1, 0, 2).reshape(128, 16 * 128),
         whhT.transpose(1, 0, 2).reshape(128, 4 * 128),
         woutT.transpose(1, 0, 2).reshape(128, P * 2),
         h0T], axis=1)

    bias4 = np.stack([np.tile(bias_s[32 * g:32 * (g + 1)], P)
                      for g in range(4)])  # [4, 128]
    ind4 = np.zeros((4, 4 * NS), np.float32)
    for g in range(4):
        ind4[g, g * NS:(g + 1) * NS] = 1.0
    spack = np.concatenate([bias4, ind4], axis=1)

    return dict(
        feedT=feedT,
        wpack=wpack.astype(BF),
        spack=spack.astype(BF),
        boutd=b_out.astype(np.float32).reshape(2, 1),
        c0T=c0T.astype(np.float32),
    )


def kernel(feed, W_ih, W_hh, b_ih, b_hh, W_out, b_out, h0, c0):
    global _nc_cache, LAST_RESULTS
    feed = np.asarray(feed, dtype=np.float32)
    W_ih = np.asarray(W_ih, dtype=np.float32)
    W_hh = np.asarray(W_hh, dtype=np.float32)
    b_ih = np.asarray(b_ih, dtype=np.float32)
    b_hh = np.asarray(b_hh, dtype=np.float32)
    W_out = np.asarray(W_out, dtype=np.float32)
    b_out = np.asarray(b_out, dtype=np.float32)
    h0 = np.asarray(h0, dtype=np.float32)
    c0 = np.asarray(c0, dtype=np.float32)

    if _nc_cache is None:
        _nc_cache = _build_nc()
    nc = _nc_cache

    in_maps = []
    for c in range(NCORES):
        rows = slice(c * BC, (c + 1) * BC)
        in_maps.append(_prep_core_inputs(
            feed[rows], W_ih, W_hh, b_ih, b_hh, W_out, b_out,
            h0[rows], c0[rows]))

    res = run_bass_kernel_spmd(nc, in_maps, core_ids=list(range(NCORES)))
    LAST_RESULTS = res

    out = np.empty((B, 2), dtype=np.float32)
    for c in range(NCORES):
        out[c * BC:(c + 1) * BC] = res.results[c]["y_out"].T
    return out
